# revision 1
# baseline (speedup 1.0000x reference)
"""Trainium2 Bass kernel for nn_MAB: MHA block (B=4, N=2048, D=256, H=8) on 8 cores.

Sharding: 8 shards = (batch b, query-half) pairs. Each core computes the full
attention + LN/FFN tail for its 1024 query rows against all 2048 keys of its
batch. All gathering happens on host; no collectives.

Numerics: fp16 matmul operands (fp32 PSUM accumulation), exp on ACT from fp32
scores, fp32 LN/FFN tail. Softmax max-subtraction is skipped (|scores|<=~1.1);
per-(row,head) denominators come free as a ones-column in the PV matmul.
"""

import numpy as np

import concourse.bass as bass
import concourse.tile as tile
from concourse import bacc, mybir
from concourse import bass_utils
from concourse.masks import make_identity

B, NQ, NK, DV, H = 4, 2048, 2048, 256, 8
HD = DV // H  # 32
NQC = 1024  # q rows per core
SCALE = 1.0 / np.sqrt(HD)
EPS = 1e-5
FP16 = mybir.dt.float16
FP32 = mybir.dt.float32


def _build():
    nc = bacc.Bacc(
        "TRN2",
        target_bir_lowering=False,
        debug=False,
        enable_asserts=False,
        num_devices=1,
    )
    d = {}
    ins = [
        ("qt", [128, 2, NQC], FP16),       # Q-shard^T  [dq(part), dq-chunk, q]
        ("kt", [128, 2, NK], FP16),        # K^T        [dq(part), dq-chunk, k]
        ("wq", [128, 2, 256], FP16),       # Wq^T       [dq(part), dq-chunk, dv]
        ("wk", [128, 2, 256], FP16),
        ("wv", [128, 2, 256], FP16),
        ("wo", [128, 2, 256], FP32),       # Wo^T
        ("bq", [128, 2], FP32),            # per-dv-channel biases, chunk-major
        ("bk", [128, 2], FP32),
        ("bo", [128, 2], FP32),
        ("bvr", [128, 256], FP32),         # bv replicated over partitions
        ("g0r", [128, 256], FP32),
        ("b0r", [128, 256], FP32),
        ("g1r", [128, 256], FP32),
        ("b1r", [128, 256], FP32),
    ]
    for name, shape, dt in ins:
        d[name] = nc.dram_tensor(name, shape, dt, kind="ExternalInput").ap()
    out_dram = nc.dram_tensor("out", [NQC, 256], FP32, kind="ExternalOutput").ap()

    with tile.TileContext(nc) as tc:
        _kernel_body(tc, d, out_dram)
    nc.compile()
    return nc


def _kernel_body(tc, d, out_dram):
    nc = tc.nc
    from contextlib import ExitStack

    ctx = ExitStack()
    with ctx:
        singles = ctx.enter_context(tc.tile_pool(name="singles", bufs=1))
        small = ctx.enter_context(tc.tile_pool(name="small", bufs=8))

        # ---- load constants / inputs to SBUF ----
        sb = {}
        for name, shape, dt in [
            ("qt", [128, 2, NQC], FP16),
            ("kt", [128, 2, NK], FP16),
            ("wq", [128, 2, 256], FP16),
            ("wk", [128, 2, 256], FP16),
            ("wv", [128, 2, 256], FP16),
            ("wo", [128, 2, 256], FP32),
            ("bq", [128, 2], FP32),
            ("bk", [128, 2], FP32),
            ("bo", [128, 2], FP32),
            ("bvr", [128, 256], FP32),
            ("g0r", [128, 256], FP32),
            ("b0r", [128, 256], FP32),
            ("g1r", [128, 256], FP32),
            ("b1r", [128, 256], FP32),
        ]:
            t = singles.tile(shape, dt, tag=name)
            nc.sync.dma_start(t[:], d[name][:])
            sb[name] = t

        ident = singles.tile([128, 128], FP32, tag="ident")
        make_identity(nc, ident[:])
        eps_sb = singles.tile([128, 1], FP32, tag="eps")
        nc.vector.memset(eps_sb[:], EPS)

        # persistent SBUF intermediates
        qp = singles.tile([128, 2, NQC], FP16, tag="qp")     # Qp^T
        kp = singles.tile([128, 2, NK], FP16, tag="kp")      # Kp^T
        vpx = singles.tile([128, 16, H, 64], FP16, tag="vpx")  # [k-part, kt, head, V|1|0]
        o_nat = singles.tile([128, 8, 512], FP32, tag="onat")  # transposed PV out per qsub
        olnT = singles.tile([128, 2, NQC], FP32, tag="olnT")
        fcT = singles.tile([128, 2, NQC], FP32, tag="fcT")
        r3T = singles.tile([128, 2, NQC], FP32, tag="r3T")
        odiv8 = singles.tile([128, 8, 256], FP32, tag="odiv8")
        r3n8 = singles.tile([128, 8, 256], FP32, tag="r3n8")
        mv8a = singles.tile([128, 8, 2], FP32, tag="mv8a")
        mv8b = singles.tile([128, 8, 2], FP32, tag="mv8b")
        rstd8a = singles.tile([128, 8], FP32, tag="rstd8a")
        rstd8b = singles.tile([128, 8], FP32, tag="rstd8b")

        nc.vector.memset(vpx[:], 0.0)
        nc.vector.memset(vpx[:, :, :, 32:33], 1.0)

        # ---- phase 1: projections ----
        with tc.tile_pool(name="prj_ps", bufs=2, space="PSUM") as prj_ps:
            # Qp^T[dv, q] and Kp^T[dv, k]
            for (wname, bname, src, dst, ncols) in [
                ("wq", "bq", "qt", qp, NQC),
                ("wk", "bk", "kt", kp, NK),
            ]:
                for dvt in range(2):
                    for qc in range(ncols // 512):
                        ps = prj_ps.tile([128, 512], FP32, tag="p512")
                        for o in range(2):
                            nc.tensor.matmul(
                                ps[:],
                                sb[wname][:, o, dvt * 128:(dvt + 1) * 128],
                                sb[src][:, o, qc * 512:(qc + 1) * 512],
                                start=(o == 0),
                                stop=(o == 1),
                            )
                        nc.vector.tensor_scalar(
                            out=dst[:, dvt, qc * 512:(qc + 1) * 512],
                            in0=ps[:],
                            scalar1=sb[bname][:, dvt:dvt + 1],
                            scalar2=None,
                            op0=mybir.AluOpType.add,
                        )
            # Vp natural [k, dv] into 64-wide head blocks with ones column
            for kt_i in range(16):
                ps = prj_ps.tile([128, 256], FP32, tag="p256")
                for o in range(2):
                    nc.tensor.matmul(
                        ps[:],
                        sb["kt"][:, o, kt_i * 128:(kt_i + 1) * 128],
                        sb["wv"][:, o, :],
                        start=(o == 0),
                        stop=(o == 1),
                    )
                nc.vector.tensor_tensor(
                    out=vpx[:, kt_i, :, 0:32],
                    in0=ps[:].rearrange("p (h e) -> p h e", h=H),
                    in1=sb["bvr"][:].rearrange("p (h e) -> p h e", h=H),
                    op=mybir.AluOpType.add,
                )

        # ---- phase 2: attention ----
        with (
            tc.tile_pool(name="sc_ps", bufs=2, space="PSUM") as sc_ps,
            tc.tile_pool(name="pv_ps", bufs=2, space="PSUM") as pv_ps,
            tc.tile_pool(name="tr_ps", bufs=2, space="PSUM") as tr_ps,
            tc.tile_pool(name="et_sb", bufs=4) as et_sb,
            tc.tile_pool(name="ev_sb", bufs=5) as ev_sb,
        ):
            for qc in range(2):  # 512-wide q chunk: attention + full tail
                for j in range(4):  # head pair (2j, 2j+1)
                    pv = pv_ps.tile([128, 512], FP32, tag="pv")
                    for kt_i in range(16):
                        sc = sc_ps.tile([128, 1024], FP32, tag="sc")
                        for hi in range(2):
                            h = 2 * j + hi
                            rp = (h % 4) * 32
                            ch = h // 4
                            nc.tensor.matmul(
                                sc[:, hi * 512:(hi + 1) * 512],
                                kp[rp:rp + 32, ch, kt_i * 128:(kt_i + 1) * 128],
                                qp[rp:rp + 32, ch, qc * 512:(qc + 1) * 512],
                                start=True,
                                stop=True,
                                tile_position=(rp, 0),
                            )
                        et = et_sb.tile([128, 1024], FP16, tag="et")
                        nc.scalar.activation(
                            out=et[:], in_=sc[:],
                            func=mybir.ActivationFunctionType.Exp,
                            scale=float(SCALE),
                        )
                        for hi in range(2):
                            h = 2 * j + hi
                            nc.tensor.matmul(
                                pv[hi * 64:(hi + 1) * 64, :],
                                vpx[:, kt_i, h, :],
                                et[:, hi * 512:(hi + 1) * 512],
                                start=(kt_i == 0),
                                stop=(kt_i == 15),
                                tile_position=(0, hi * 64),
                            )
                    # evict PV accumulator and transpose to natural layout
                    pvs = ev_sb.tile([128, 512], FP32, tag="pvs")
                    nc.vector.tensor_copy(out=pvs[:], in_=pv[:])
                    for qs in range(4):
                        trp = tr_ps.tile([128, 128], FP32, tag="tr")
                        nc.tensor.transpose(trp[:], pvs[:, qs * 128:(qs + 1) * 128], ident[:])
                        qsub = qc * 4 + qs
                        nc.vector.tensor_copy(
                            out=o_nat[:, qsub, j * 128:(j + 1) * 128], in_=trp[:]
                        )

                # per-qc pre-LN work (pure DVE, no ACT): overlaps with the
                # next q-chunk's attention without blocking the exp queue
                for qs in range(4):
                    qsub = qc * 4 + qs
                    rd = small.tile([128, 8], FP32, tag="rd")
                    nc.vector.reciprocal(out=rd[:], in_=o_nat[:, qsub, 32::64])
                    for h in range(H):
                        cb = (h // 2) * 128 + (h % 2) * 64
                        nc.vector.tensor_scalar(
                            out=odiv8[:, qsub, h * 32:(h + 1) * 32],
                            in0=o_nat[:, qsub, cb:cb + 32],
                            scalar1=rd[:, h:h + 1],
                            scalar2=None,
                            op0=mybir.AluOpType.mult,
                        )
                    stats = small.tile([128, 6], FP32, tag="stats")
                    nc.vector.bn_stats(out=stats[:], in_=odiv8[:, qsub, :])
                    nc.vector.bn_aggr(out=mv8a[:, qsub, :], in_=stats[:])

            # ---- tail (LN variances batched: Scalar sees exactly two Sqrt
            # ops after all exps -> no table thrash) ----
            _batched_rstd(nc, small, mv8a, rstd8a, eps_sb)
            for qsub in range(8):
                oln = _ln_apply(nc, ev_sb, odiv8[:, qsub, :],
                                mv8a[:, qsub, 0:1], rstd8a[:, qsub:qsub + 1],
                                sb["g0r"], sb["b0r"])
                for dvt in range(2):
                    trp = tr_ps.tile([128, 128], FP32, tag="tr")
                    nc.tensor.transpose(trp[:], oln[:, dvt * 128:(dvt + 1) * 128], ident[:])
                    nc.vector.tensor_copy(
                        out=olnT[:, dvt, qsub * 128:(qsub + 1) * 128], in_=trp[:]
                    )

            # ---- fc_o + relu + residual ----
            for qc in range(2):
                for dvt in range(2):
                    ps = tr_ps.tile([128, 512], FP32, tag="tr")
                    for o in range(2):
                        nc.tensor.matmul(
                            ps[:],
                            sb["wo"][:, o, dvt * 128:(dvt + 1) * 128],
                            olnT[:, o, qc * 512:(qc + 1) * 512],
                            start=(o == 0),
                            stop=(o == 1),
                        )
                    nc.vector.tensor_scalar(
                        out=fcT[:, dvt, qc * 512:(qc + 1) * 512],
                        in0=ps[:],
                        scalar1=sb["bo"][:, dvt:dvt + 1],
                        scalar2=0.0,
                        op0=mybir.AluOpType.add,
                        op1=mybir.AluOpType.max,
                    )
            nc.vector.tensor_tensor(
                out=r3T[:], in0=olnT[:], in1=fcT[:], op=mybir.AluOpType.add
            )

            # ---- transpose back + LN1 (batched variance) + store ----
            for qsub in range(8):
                for dvt in range(2):
                    trp = tr_ps.tile([128, 128], FP32, tag="tr")
                    nc.tensor.transpose(
                        trp[:], r3T[:, dvt, qsub * 128:(qsub + 1) * 128], ident[:]
                    )
                    nc.vector.tensor_copy(
                        out=r3n8[:, qsub, dvt * 128:(dvt + 1) * 128], in_=trp[:]
                    )
                stats = small.tile([128, 6], FP32, tag="stats")
                nc.vector.bn_stats(out=stats[:], in_=r3n8[:, qsub, :])
                nc.vector.bn_aggr(out=mv8b[:, qsub, :], in_=stats[:])
            _batched_rstd(nc, small, mv8b, rstd8b, eps_sb)
            for qsub in range(8):
                fin = _ln_apply(nc, ev_sb, r3n8[:, qsub, :],
                                mv8b[:, qsub, 0:1], rstd8b[:, qsub:qsub + 1],
                                sb["g1r"], sb["b1r"])
                nc.sync.dma_start(out_dram[qsub * 128:(qsub + 1) * 128, :], fin[:])


def _batched_rstd(nc, small, mv8, rstd8, eps_sb):
    """rstd for 8 rows at once: one Sqrt ACT + one DVE reciprocal."""
    std8 = small.tile([128, 8], FP32, tag="std8")
    nc.scalar.activation(
        out=std8[:], in_=mv8[:, :, 1],
        func=mybir.ActivationFunctionType.Sqrt, bias=eps_sb[:],
    )
    nc.vector.reciprocal(out=rstd8[:], in_=std8[:])


def _ln_apply(nc, ev_sb, x, mean, rstd, g_rep, b_rep):
    xn = ev_sb.tile([128, 256], FP32, tag="xn")
    nc.vector.tensor_scalar(
        out=xn[:], in0=x[:],
        scalar1=mean, scalar2=rstd,
        op0=mybir.AluOpType.subtract, op1=mybir.AluOpType.mult,
    )
    nc.vector.tensor_tensor(out=xn[:], in0=xn[:], in1=g_rep[:], op=mybir.AluOpType.mult)
    nc.vector.tensor_tensor(out=xn[:], in0=xn[:], in1=b_rep[:], op=mybir.AluOpType.add)
    return xn


_NC = None


def _get_nc():
    global _NC
    if _NC is None:
        _NC = _build()
    return _NC


def _chunk_major(v):
    # [256] channel vector -> [128, 2] where [p, o] = v[o*128+p]
    return np.ascontiguousarray(v.reshape(2, 128).T.astype(np.float32))


def _prep_inputs(Q, K, Wq, bq, Wk, bk, Wv, bv, Wo, bo, g0, b0, g1, b1):
    def t_chunks(m, dt):
        # [256, n] -> [128, 2, n]: row d = o*128+p goes to [p, o, :]
        return np.ascontiguousarray(
            m.reshape(2, 128, m.shape[1]).transpose(1, 0, 2).astype(dt)
        )

    wq_t = t_chunks(Wq.T, np.float16)
    wk_t = t_chunks(Wk.T, np.float16)
    wv_t = t_chunks(Wv.T, np.float16)
    wo_t = t_chunks(Wo.T, np.float32)
    rep = lambda v: np.ascontiguousarray(
        np.broadcast_to(v.astype(np.float32), (128, 256))
    )
    common = {
        "wq": wq_t, "wk": wk_t, "wv": wv_t, "wo": wo_t,
        "bq": _chunk_major(bq), "bk": _chunk_major(bk), "bo": _chunk_major(bo),
        "bvr": rep(bv), "g0r": rep(g0), "b0r": rep(b0),
        "g1r": rep(g1), "b1r": rep(b1),
    }
    in_maps = []
    for c in range(8):
        b, qh = c // 2, c % 2
        qt = t_chunks(Q[b, qh * NQC:(qh + 1) * NQC, :].T, np.float16)
        kt = t_chunks(K[b].T, np.float16)
        in_maps.append({"qt": qt, "kt": kt, **common})
    return in_maps


def _run(inputs, trace=False):
    nc = _get_nc()
    in_maps = _prep_inputs(**inputs)
    res = bass_utils.run_bass_kernel_spmd(
        nc, in_maps, core_ids=list(range(8)), trace=trace
    )
    out = np.empty((B, NQ, DV), np.float32)
    for c in range(8):
        b, qh = c // 2, c % 2
        out[b, qh * NQC:(qh + 1) * NQC, :] = res.results[c]["out"]
    return out, res


def kernel(**inputs):
    inputs = {k: np.asarray(v) for k, v in inputs.items()}
    out, _ = _run(inputs, trace=False)
    return out

